# revision 3
# baseline (speedup 1.0000x reference)
"""Trainium2 Bass kernel for a 2-layer LSTM (H=10, IN=2, T=80, B=32768) + MLP head.

Strategy (data-parallel over batch across 8 NeuronCores):

Launch 1 (LSTM): a "wavefront" over the two LSTM layers — wave s computes
layer0 @ t=s and layer1 @ t=s-1 in the same pass.  The recurrent state is
kept in a [23, batch] "stack" (h0:10, h1:10, x:2, ones:1) with hid on
partitions; each 128-batch chunk of the stack is the *stationary* matmul
operand (lhsT), and the combined weight matrix [23, 80] is streamed, so the
gates come out [128 batch, 80 gates] with batch on partitions — which keeps
the sigmoid/tanh + c/h elementwise chain running at full 128-lane width.
h is returned to the hid-on-partition stack with a DVE 32x32 block
transpose + an SBUF->SBUF block-gather DMA.  Two independent batch streams
per core software-pipeline the per-wave critical path.

Launch 2 (MLP head): the reference's raw reshape [T,B,H] -> [B, 800] mixes
batch across cores, so z=tanh(y1) is returned to the host, resharded by
output row, and a small second kernel computes sigmoid(Z@W1.T+b1)@W2.T+b2.
"""

import numpy as np
from contextlib import ExitStack

import concourse.bass as bass
import concourse.tile as tile
from concourse import mybir
from concourse.bass_utils import run_bass_kernel_spmd
from concourse.vector_clock import ScopedClock

F32 = mybir.dt.float32
AF = mybir.ActivationFunctionType

SEQ, B, IN, H = 80, 32768, 2, 10
NCORES = 8
BC = B // NCORES          # 4096 batch per core
NSTREAM = 2
SB = BC // NSTREAM        # 2048 batch per stream
NCH = SB // 128           # 16 chunks of 128 batch per stream
CW = 2 * H                # 20 state columns (h0 | h1)
KR = 23                   # stack rows: h0(10) h1(10) x(2) ones(1)
OUT_LEN, OUT_SIZE = 20, 2
K2 = 7                    # MLP contraction chunks: 896 = 7*128 (>= 801)

_WS = [0]


class PatchedTileContext(tile.TileContext):
    """This walrus build allows only ONE sem-wait per instruction; hoist
    extra waits onto same-engine NoOps, and split the tail drain's waits."""

    def _split_multi_waits(self, ordered):
        for bb_name, insts in ordered.items():
            out = []
            for inst in insts:
                si = inst.sync_info
                if si is not None and si.on_wait and len(si.on_wait) > 1:
                    waits = list(si.on_wait)
                    for w in waits[:-1]:
                        _WS[0] += 1
                        nop = mybir.InstNoOp(
                            name=f"I-wsplit-{_WS[0]}", ins=[], outs=[]
                        )
                        nop.engine = inst.engine
                        nop.sync_info = mybir.SyncInfo(on_wait=[w], on_update=[])
                        self.nc.register_instruction(nop, overwrite=True)
                        out.append(nop)
                    inst.sync_info = mybir.SyncInfo(
                        on_wait=[waits[-1]], on_update=list(si.on_update or [])
                    )
                out.append(inst)
            ordered[bb_name] = out
        return ordered

    def _lower_ordered_insts(self, ordered):
        ordered = self._split_multi_waits(ordered)
        return super()._lower_ordered_insts(ordered)

    def _drain_and_barrier(self, tick_clock, wait_clock):
        nc = self.nc
        drain_inst = nc.sync.drain()
        wait_clock.add_sem_waits(
            drain_inst.ins, ScopedClock({None: tick_clock.global_clock})
        )
        si = drain_inst.ins.sync_info
        if si is not None and si.on_wait and len(si.on_wait) > 1:
            waits = list(si.on_wait)
            drain_inst.ins.sync_info = mybir.SyncInfo(
                on_wait=[waits[0]], on_update=list(si.on_update or [])
            )
            for w in waits[1:]:
                nop = nc.sync.nop(nofuse=True)
                nop.ins.sync_info = mybir.SyncInfo(on_wait=[w], on_update=[])
        nc.all_engine_barrier()
        popped = nc._tile_sem_poison_stack.pop()
        assert popped is self._sem_poison
        nc.clear_and_free_semaphores(list(self.sems.allocated().values()))
        nc.all_engine_barrier()


def build_lstm(seq=SEQ):
    """Wavefront 2-layer LSTM; outputs z = tanh(h1) per timestep."""
    nc = bass.Bass("TRN2")
    xT_d = nc.declare_dram_parameter("xT", [seq, IN, BC], F32, isOutput=False)
    h0T_d = nc.declare_dram_parameter("h0T", [CW, BC], F32, isOutput=False)
    c0p_d = nc.declare_dram_parameter("c0p", [128, BC // 128, CW], F32, isOutput=False)
    wf_d = nc.declare_dram_parameter("wfirst", [KR, 40], F32, isOutput=False)
    wm_d = nc.declare_dram_parameter("wmid", [KR, 80], F32, isOutput=False)
    wl_d = nc.declare_dram_parameter("wlast", [KR, 40], F32, isOutput=False)
    ones_d = nc.declare_dram_parameter("ones", [1, BC], F32, isOutput=False)
    z_d = nc.declare_dram_parameter("z", [seq, 128, (BC // 128) * H], F32, isOutput=True)

    with PatchedTileContext(nc) as tc, ExitStack() as ctx:
        const = ctx.enter_context(tc.tile_pool(name="const", bufs=1))
        state = ctx.enter_context(tc.tile_pool(name="state", bufs=1))
        psum = ctx.enter_context(tc.tile_pool(name="psum", bufs=1, space="PSUM"))
        work = ctx.enter_context(tc.tile_pool(name="work", bufs=3))

        wfirst = const.tile([KR, 40], F32, name="wfirst_t")
        nc.sync.dma_start(wfirst[:], wf_d[:])
        wmid = const.tile([KR, 80], F32, name="wmid_t")
        nc.sync.dma_start(wmid[:], wm_d[:])
        wlast = const.tile([KR, 40], F32, name="wlast_t")
        nc.sync.dma_start(wlast[:], wl_d[:])

        stacks, ctiles, htmps, htmpTs = [], [], [], []
        for st in range(NSTREAM):
            bsl = slice(st * SB, (st + 1) * SB)
            bufs2 = []
            for bi in range(2):
                stk = state.tile([KR, SB], F32, name=f"stack{st}_{bi}")
                nc.sync.dma_start(stk[22:23, :], ones_d[:, bsl])
                nc.sync.dma_start(stk[0:CW, :], h0T_d[:, bsl])
                bufs2.append(stk)
            stacks.append(bufs2)
            ct = state.tile([128, NCH, CW], F32, name=f"ctile{st}")
            nc.sync.dma_start(ct[:], c0p_d[:, st * NCH:(st + 1) * NCH, :])
            ctiles.append(ct)
            ht = state.tile([128, NCH, 32], F32, name=f"htmp{st}")
            nc.gpsimd.memset(ht[:], 0.0)
            htmps.append(ht)
            htT = state.tile([128, NCH, 32], F32, name=f"htmpT{st}")
            httiles = htT
            htmpTs.append(httiles)

        for s in range(seq + 1):
            L0 = s < seq
            L1 = s >= 1
            if L0 and L1:
                wt, gw, coff, cw = wmid, 20, 0, 20
            elif L0:
                wt, gw, coff, cw = wfirst, 10, 0, 10
            else:
                wt, gw, coff, cw = wlast, 10, 10, 10
            gwall = 4 * gw

            for st in range(NSTREAM):
                bsl = slice(st * SB, (st + 1) * SB)
                stk = stacks[st][s % 2]
                nxt = stacks[st][(s + 1) % 2]
                ct = ctiles[st]
                ht = htmps[st]
                htT = htmpTs[st]

                if L0:
                    nc.sync.dma_start(stk[20:22, :], xT_d[s, :, bsl])

                gates = psum.tile([128, NCH, 128], F32, name=f"gates{st}", tag=f"ps{st}")
                for c in range(NCH):
                    nc.tensor.matmul(
                        gates[:, c, 0:gwall],
                        stk[:, c * 128:(c + 1) * 128],
                        wt[:],
                        start=True,
                        stop=True,
                    )

                # gate col order per chunk: [i | f | o | g], each gw wide,
                # layer0 then layer1 inside each group when both active.
                sig = work.tile([128, NCH, 60], F32, name=f"sig{st}", tag=f"sig{st}")
                nc.scalar.activation(sig[:, :, 0:3 * gw], gates[:, :, 0:3 * gw], AF.Sigmoid)
                gt = work.tile([128, NCH, 20], F32, name=f"gt{st}", tag=f"gt{st}")
                nc.scalar.activation(gt[:, :, 0:gw], gates[:, :, 3 * gw:4 * gw], AF.Tanh)

                pt = work.tile([128, NCH, 20], F32, name=f"pt{st}", tag=f"pt{st}")
                nc.vector.tensor_mul(pt[:, :, 0:cw], sig[:, :, 0:gw], gt[:, :, 0:gw])
                qt = work.tile([128, NCH, 20], F32, name=f"qt{st}", tag=f"qt{st}")
                nc.gpsimd.tensor_mul(qt[:, :, 0:cw], sig[:, :, gw:2 * gw], ct[:, :, coff:coff + cw])
                nc.gpsimd.tensor_add(ct[:, :, coff:coff + cw], pt[:, :, 0:cw], qt[:, :, 0:cw])

                tct = work.tile([128, NCH, 20], F32, name=f"tct{st}", tag=f"tct{st}")
                nc.scalar.activation(tct[:, :, 0:cw], ct[:, :, coff:coff + cw], AF.Tanh)
                nc.vector.tensor_mul(ht[:, :, coff:coff + cw], sig[:, :, 2 * gw:3 * gw], tct[:, :, 0:cw])

                if L1:
                    zt = work.tile([128, NCH, H], F32, name=f"zt{st}", tag=f"zt{st}")
                    nc.scalar.activation(zt[:], ht[:, :, 10:20], AF.Tanh)
                    nc.sync.dma_start(
                        z_d[s - 1].rearrange("p (c h) -> p c h", h=H)[:, st * NCH:(st + 1) * NCH, :],
                        zt[:],
                    )

                if s < seq:
                    nc.vector.transpose(
                        htT[:].rearrange("p c k -> p (c k)"),
                        ht[:].rearrange("p c k -> p (c k)"),
                    )
                    k0, kn = (0, 10) if not L1 else (0, 20)
                    for i in range(4):
                        src = htT[32 * i + k0:32 * i + k0 + kn, :, :]
                        dst = nxt[k0:k0 + kn, :].rearrange(
                            "k (c i b) -> k c i b", c=NCH, i=4, b=32
                        )[:, :, i, :]
                        nc.sync.dma_start(dst, src)
    return nc


def build_mlp():
    """out2 = sigmoid(Z2 @ W1.T + b1) @ W2.T + b2 for one row-shard."""
    nc = bass.Bass("TRN2")
    z2t_d = nc.declare_dram_parameter("z2t", [K2, 128, BC], F32, isOutput=False)
    w1b_d = nc.declare_dram_parameter("w1b", [K2, 128, H], F32, isOutput=False)
    w2b_d = nc.declare_dram_parameter("w2b", [H + 1, 40], F32, isOutput=False)
    ones_d = nc.declare_dram_parameter("ones", [1, BC], F32, isOutput=False)
    out_d = nc.declare_dram_parameter("out2", [40, BC], F32, isOutput=True)

    with PatchedTileContext(nc) as tc, ExitStack() as ctx:
        const = ctx.enter_context(tc.tile_pool(name="const", bufs=1))
        pool = ctx.enter_context(tc.tile_pool(name="pool", bufs=3))
        ps = ctx.enter_context(tc.tile_pool(name="ps", bufs=2, space="PSUM"))

        w1 = const.tile([128, K2, H], F32, name="w1_t")
        nc.sync.dma_start(w1[:], w1b_d[:].rearrange("k p h -> p k h"))
        w2 = const.tile([H + 1, 40], F32, name="w2_t")
        nc.sync.dma_start(w2[:], w2b_d[:])
        sstack = const.tile([H + 1, BC], F32, name="sstack")
        nc.sync.dma_start(sstack[H:H + 1, :], ones_d[:])

        NCOL = BC // 512
        for col in range(NCOL):
            zz = pool.tile([128, K2, 512], F32, name="zz", tag="zz")
            nc.sync.dma_start(
                zz[:], z2t_d[:, :, col * 512:(col + 1) * 512].rearrange("k p n -> p k n")
            )
            a1 = ps.tile([H, 512], F32, name="a1", tag="a1")
            for k in range(K2):
                nc.tensor.matmul(
                    a1[:], w1[:, k, :], zz[:, k, :], start=(k == 0), stop=(k == K2 - 1)
                )
            nc.scalar.activation(sstack[0:H, col * 512:(col + 1) * 512], a1[:], AF.Sigmoid)
        for col in range(NCOL):
            a2 = ps.tile([40, 512], F32, name="a2", tag="a2")
            nc.tensor.matmul(a2[:], w2[:], sstack[:, col * 512:(col + 1) * 512], start=True, stop=True)
            ot = pool.tile([40, 512], F32, name="ot", tag="ot")
            nc.vector.tensor_copy(ot[:], a2[:])
            nc.sync.dma_start(out_d[:, col * 512:(col + 1) * 512], ot[:])
    return nc


def _build_weight_mats(Wih0, Whh0, bih0, bhh0, Wih1, Whh1, bih1, bhh1):
    b0 = (bih0 + bhh0).astype(np.float32)
    b1 = (bih1 + bhh1).astype(np.float32)
    rows = {"i": slice(0, 10), "f": slice(10, 20), "g": slice(20, 30), "o": slice(30, 40)}
    order = ["i", "f", "o", "g"]
    wmid = np.zeros((KR, 80), np.float32)
    wfirst = np.zeros((KR, 40), np.float32)
    wlast = np.zeros((KR, 40), np.float32)
    for bi, gtp in enumerate(order):
        gr = rows[gtp]
        c0 = slice(bi * 20, bi * 20 + 10)
        c1 = slice(bi * 20 + 10, bi * 20 + 20)
        wmid[0:10, c0] = Whh0[gr, :].T
        wmid[20:22, c0] = Wih0[gr, :].T
        wmid[22, c0] = b0[gr]
        wmid[0:10, c1] = Wih1[gr, :].T
        wmid[10:20, c1] = Whh1[gr, :].T
        wmid[22, c1] = b1[gr]
        cs = slice(bi * 10, bi * 10 + 10)
        wfirst[0:10, cs] = Whh0[gr, :].T
        wfirst[20:22, cs] = Wih0[gr, :].T
        wfirst[22, cs] = b0[gr]
        wlast[0:10, cs] = Wih1[gr, :].T
        wlast[10:20, cs] = Whh1[gr, :].T
        wlast[22, cs] = b1[gr]
    return wfirst, wmid, wlast


_CACHE = {}


def _get_lstm():
    if "lstm" not in _CACHE:
        _CACHE["lstm"] = build_lstm()
    return _CACHE["lstm"]


def _get_mlp():
    if "mlp" not in _CACHE:
        _CACHE["mlp"] = build_mlp()
    return _CACHE["mlp"]


def kernel(x, h0, c0, Wih0, Whh0, bih0, bhh0, Wih1, Whh1, bih1, bhh1, W1, b1, W2, b2):
    x = np.asarray(x, np.float32)
    h0 = np.asarray(h0, np.float32)
    c0 = np.asarray(c0, np.float32)
    wfirst, wmid, wlast = _build_weight_mats(
        np.asarray(Wih0, np.float32), np.asarray(Whh0, np.float32),
        np.asarray(bih0, np.float32), np.asarray(bhh0, np.float32),
        np.asarray(Wih1, np.float32), np.asarray(Whh1, np.float32),
        np.asarray(bih1, np.float32), np.asarray(bhh1, np.float32),
    )
    core_ids = list(range(NCORES))

    in_maps = []
    for j in core_ids:
        bsl = slice(j * BC, (j + 1) * BC)
        xT = np.ascontiguousarray(x[:, bsl, :].transpose(0, 2, 1))
        h0T = np.concatenate([h0[0, bsl, :], h0[1, bsl, :]], axis=1).T
        h0T = np.ascontiguousarray(h0T)
        cc = np.concatenate([c0[0, bsl, :], c0[1, bsl, :]], axis=1)  # [BC, 20]
        c0p = np.ascontiguousarray(cc.reshape(BC // 128, 128, CW).transpose(1, 0, 2))
        in_maps.append({
            "xT": xT, "h0T": h0T, "c0p": c0p,
            "wfirst": wfirst, "wmid": wmid, "wlast": wlast,
            "ones": np.ones((1, BC), np.float32),
        })

    res1 = run_bass_kernel_spmd(_get_lstm(), in_maps, core_ids).results

    # z dram layout per core: [t, p, c*H + h] with local batch b = 128*c + p
    z_cores = []
    for j in core_ids:
        zj = res1[j]["z"].reshape(SEQ, 128, BC // 128, H).transpose(0, 2, 1, 3)
        z_cores.append(zj.reshape(SEQ, BC, H))
    z_global = np.concatenate(z_cores, axis=1)          # [T, B, H]
    Z2 = np.ascontiguousarray(z_global).reshape(B, SEQ * H)

    w1b = np.zeros((K2 * 128, H), np.float32)
    w1b[0:SEQ * H, :] = np.asarray(W1, np.float32).T
    w1b[SEQ * H, :] = np.asarray(b1, np.float32)
    w1b = w1b.reshape(K2, 128, H)
    w2b = np.zeros((H + 1, 40), np.float32)
    w2b[0:H, :] = np.asarray(W2, np.float32).T
    w2b[H, :] = np.asarray(b2, np.float32)

    in_maps2 = []
    for j in core_ids:
        rows = slice(j * BC, (j + 1) * BC)
        z2t = np.zeros((K2 * 128, BC), np.float32)
        z2t[0:SEQ * H, :] = Z2[rows, :].T
        z2t[SEQ * H, :] = 1.0
        in_maps2.append({
            "z2t": np.ascontiguousarray(z2t.reshape(K2, 128, BC)),
            "w1b": w1b, "w2b": w2b,
            "ones": np.ones((1, BC), np.float32),
        })

    res2 = run_bass_kernel_spmd(_get_mlp(), in_maps2, core_ids).results
    out2 = np.concatenate([res2[j]["out2"] for j in core_ids], axis=1)  # [40, B]
    out = np.ascontiguousarray(out2.T).reshape(OUT_LEN, B, OUT_SIZE)
    return out


# revision 16
# speedup vs baseline: 52103.5994x; 52103.5994x over previous
"""Trainium2 Bass kernel for a 2-layer LSTM (H=10, IN=2, T=80, B=32768) + MLP head.

Strategy (data-parallel over batch across 8 NeuronCores):

Launch 1 (LSTM): a "wavefront" over the two LSTM layers — wave s computes
layer0 @ t=s and layer1 @ t=s-1 in the same pass.  All elementwise state
(c, h, gates) lives batch-on-partition so ACT/DVE run at full 128-lane
width.  The recurrent matmul input is produced by a DVE 32x32 block
transpose of an "htmp" tile whose 32 free columns are [h0(10) h1(10) x(2)
ones(1) pad]; the transposed tile htT directly provides, for each
(32-batch block i, 128-batch chunk c), a [23, 32] stationary operand at
partition base 32*i.  Per chunk, four row+col-tiled matmuls at
tile_position (32i, 32i) against a 4x-replicated weight matrix produce
gates [128 batch, 80] in PSUM — no gather DMAs, no per-chunk weight
reloads from DRAM, and x is injected by one cheap DMA per wave into
htmp's columns.  Two independent batch streams per core software-pipeline
the per-wave critical path; z = tanh(h1) accumulates in SBUF and flushes
every 8 waves.

Launch 2 (MLP head): the reference's raw reshape [T,B,H] -> [B, 800] mixes
batch across cores, so z is returned to the host, resharded by output row,
and a small second kernel computes sigmoid(Z@W1.T+b1)@W2.T+b2.
"""

import numpy as np
import ml_dtypes
from contextlib import ExitStack

import concourse.bass as bass
import concourse.tile as tile
from concourse import mybir
from concourse.bass_utils import run_bass_kernel_spmd
from concourse.vector_clock import ScopedClock

F32 = mybir.dt.float32
BF16 = mybir.dt.bfloat16
F32R = mybir.dt.float32r
NPBF = ml_dtypes.bfloat16
AF = mybir.ActivationFunctionType

SEQ, B, IN, H = 80, 32768, 2, 10
NCORES = 8
BC = B // NCORES          # 4096 batch per core
NSTREAM = 4
SB = BC // NSTREAM        # 2048 batch per stream
NCH = SB // 128           # 16 chunks of 128 batch per stream
CW = 2 * H                # 20 state columns (h0 | h1)
KR = 23                   # lhsT rows: h0(10) h1(10) x(2) ones(1)
ZB = 8                    # z flush batching (waves)
OUT_LEN, OUT_SIZE = 20, 2
K2 = 7                    # MLP contraction chunks: 896 = 7*128 (>= 801)

_WS = [0]


class PatchedTileContext(tile.TileContext):
    """This walrus build allows only ONE sem-wait per instruction; hoist
    extra waits onto same-engine NoOps, and split the tail drain's waits."""

    def _split_multi_waits(self, ordered):
        for bb_name, insts in ordered.items():
            out = []
            for inst in insts:
                si = inst.sync_info
                if si is not None and si.on_wait and len(si.on_wait) > 1:
                    waits = list(si.on_wait)
                    for w in waits[:-1]:
                        _WS[0] += 1
                        nop = mybir.InstNoOp(
                            name=f"I-wsplit-{_WS[0]}", ins=[], outs=[]
                        )
                        nop.engine = inst.engine
                        nop.sync_info = mybir.SyncInfo(on_wait=[w], on_update=[])
                        self.nc.register_instruction(nop, overwrite=True)
                        out.append(nop)
                    inst.sync_info = mybir.SyncInfo(
                        on_wait=[waits[-1]], on_update=list(si.on_update or [])
                    )
                out.append(inst)
            ordered[bb_name] = out
        return ordered

    def _lower_ordered_insts(self, ordered):
        ordered = self._split_multi_waits(ordered)
        return super()._lower_ordered_insts(ordered)

    def _drain_and_barrier(self, tick_clock, wait_clock):
        nc = self.nc
        drain_inst = nc.sync.drain()
        wait_clock.add_sem_waits(
            drain_inst.ins, ScopedClock({None: tick_clock.global_clock})
        )
        si = drain_inst.ins.sync_info
        if si is not None and si.on_wait and len(si.on_wait) > 1:
            waits = list(si.on_wait)
            drain_inst.ins.sync_info = mybir.SyncInfo(
                on_wait=[waits[0]], on_update=list(si.on_update or [])
            )
            for w in waits[1:]:
                nop = nc.sync.nop(nofuse=True)
                nop.ins.sync_info = mybir.SyncInfo(on_wait=[w], on_update=[])
        nc.all_engine_barrier()
        popped = nc._tile_sem_poison_stack.pop()
        assert popped is self._sem_poison
        nc.clear_and_free_semaphores(list(self.sems.allocated().values()))
        nc.all_engine_barrier()


def build_lstm(seq=SEQ, nstream=NSTREAM):
    """Wavefront 2-layer LSTM; outputs z = tanh(h1) per timestep.

    nstream "quarter" streams carry gates/sigmoid/cell updates; pairs of
    quarters share c/h state tiles so tanh-c, z and the transpose run at
    double granularity (half the per-instruction overhead on ACT/DVE).
    """
    nq = nstream               # quarter streams
    npair = max(1, nq // 2)
    nchq = BC // 128 // nq     # chunks per quarter
    nchp = BC // 128 // npair  # chunks per pair
    nc = bass.Bass("TRN2")
    xT_d = nc.declare_dram_parameter("xT", [seq, 128, BC // 128, IN], BF16, isOutput=False)
    h0p_d = nc.declare_dram_parameter("h0p", [128, BC // 128, CW], BF16, isOutput=False)
    c0p_d = nc.declare_dram_parameter("c0p", [128, BC // 128, CW], F32, isOutput=False)
    wf_d = nc.declare_dram_parameter("wfirst", [128, 40], BF16, isOutput=False)
    wm_d = nc.declare_dram_parameter("wmid", [128, 80], BF16, isOutput=False)
    wl_d = nc.declare_dram_parameter("wlast", [128, 40], BF16, isOutput=False)
    z_d = nc.declare_dram_parameter("z", [seq, 128, (BC // 128) * H], BF16, isOutput=True)

    with PatchedTileContext(nc) as tc, ExitStack() as ctx:
        const = ctx.enter_context(tc.tile_pool(name="const", bufs=1))
        state = ctx.enter_context(tc.tile_pool(name="state", bufs=1))
        psum = ctx.enter_context(tc.tile_pool(name="psum", bufs=1, space="PSUM"))
        work = ctx.enter_context(tc.tile_pool(name="work", bufs=3))

        wfirst = const.tile([128, 40], BF16, name="wfirst_t")
        nc.sync.dma_start(wfirst[:], wf_d[:])
        wmid = const.tile([128, 80], BF16, name="wmid_t")
        nc.sync.dma_start(wmid[:], wm_d[:])
        wlast = const.tile([128, 40], BF16, name="wlast_t")
        nc.sync.dma_start(wlast[:], wl_d[:])

        ctiles, htmps, htTs, zaccs = [], [], [], []
        for pr in range(npair):
            csl = slice(pr * nchp, (pr + 1) * nchp)
            ct = state.tile([128, nchp, CW], F32, name=f"ctile{pr}")
            nc.sync.dma_start(ct[:], c0p_d[:, csl, :])
            ctiles.append(ct)
            ht = state.tile([128, nchp, 32], BF16, name=f"htmp{pr}")
            nc.gpsimd.memset(ht[:, :, 22:32], 0.0)
            nc.gpsimd.memset(ht[:, :, 22:23], 1.0)
            nc.sync.dma_start(ht[:, :, 0:CW], h0p_d[:, csl, :])
            htmps.append(ht)
            htT = state.tile([128, nchp, 32], BF16, name=f"htT{pr}")
            htTs.append(htT)
            za = state.tile([128, ZB, nchp, H], BF16, name=f"zacc{pr}")
            zaccs.append(za)

        def x_dma(s, pr):
            # xT is host-packed [seq, p, c, k] with per-core batch b = 128*c + p.
            nc.sync.dma_start(
                htmps[pr][:, :, 20:22],
                xT_d[s, :, pr * nchp:(pr + 1) * nchp, :],
            )

        for pr in range(npair):
            x_dma(0, pr)
            nc.vector.transpose(
                htTs[pr][:].rearrange("p c k -> p (c k)"),
                htmps[pr][:].rearrange("p c k -> p (c k)"),
            )

        for s in range(seq + 1):
            L0 = s < seq
            L1 = s >= 1
            if L0 and L1:
                wt, gw, coff, cw = wmid, 20, 0, 20
            elif L0:
                wt, gw, coff, cw = wfirst, 10, 0, 10
            else:
                wt, gw, coff, cw = wlast, 10, 10, 10
            gwall = 4 * gw

            # quarter q covers chunks [q*nchq, (q+1)*nchq) = pair q//2,
            # half q%2.  Emission is phase-ordered so the in-order engine
            # queues interleave the streams' dependency chains.
            if s + 1 < seq:
                for pr in range(npair):
                    x_dma(s + 1, pr)
            gates_l, sig_l, tct_l = [], [], []
            for q in range(nq):
                pr, qh = q // 2, q % 2
                gates = psum.tile([128, nchq, 128], F32, name=f"gates{q}", tag=f"ps{q}")
                gates_l.append(gates)
                for c in range(nchq):
                    cc = qh * nchq + c  # chunk within pair tile
                    for i in range(4):
                        nc.tensor.matmul(
                            gates[32 * i:32 * i + 32, c, 0:gwall],
                            htTs[pr][32 * i:32 * i + KR, cc, :],
                            wt[32 * i:32 * i + KR, 0:gwall],
                            start=True,
                            stop=True,
                            tile_position=(32 * i, 32 * i),
                        )
            # gate col order per chunk: [i | f | o | g2], each gw wide, layer0
            # then layer1 inside each group when both active.  The g-columns
            # of the weight matrix are pre-doubled so sigmoid gives
            # g~ = (tanh(g)+1)/2 and i*g = 2*i*g~ - i.
            def sigma(q):
                sig = work.tile([128, nchq, 80], F32, name=f"sig{q}", tag=f"sig{q}")
                sig_l.append(sig)
                nc.scalar.activation(sig[:, :, 0:4 * gw], gates_l[q][:, :, 0:4 * gw], AF.Sigmoid)

            def pool_chain(q):
                # c' = f*c + i*(2*g~ - 1), grouped per quarter so each
                # quarter's cell update completes as soon as its sigmoid does
                pr, qh = q // 2, q % 2
                hsl = slice(qh * nchq, (qh + 1) * nchq)
                sig = sig_l[q]
                mt = work.tile([128, nchq, 20], F32, name=f"mt{q}", tag=f"mt{q}")
                nc.gpsimd.tensor_mul(mt[:, :, 0:cw], sig[:, :, 0:gw], sig[:, :, 3 * gw:4 * gw])
                qt = work.tile([128, nchq, 20], F32, name=f"qt{q}", tag=f"qt{q}")
                nc.gpsimd.tensor_mul(
                    qt[:, :, 0:cw], sig[:, :, gw:2 * gw], ctiles[pr][:, hsl, coff:coff + cw]
                )
                pt = work.tile([128, nchq, 20], F32, name=f"pt{q}", tag=f"pt{q}")
                nc.vector.scalar_tensor_tensor(
                    pt[:, :, 0:cw], mt[:, :, 0:cw], 2.0, sig[:, :, 0:gw],
                    op0=mybir.AluOpType.mult, op1=mybir.AluOpType.subtract,
                )
                nc.gpsimd.tensor_add(
                    ctiles[pr][:, hsl, coff:coff + cw], pt[:, :, 0:cw], qt[:, :, 0:cw]
                )

            def tanh_c(pr):
                tct = work.tile([128, nchp, 20], F32, name=f"tct{pr}", tag=f"tct{pr}")
                tct_l.append(tct)
                nc.scalar.activation(tct[:, :, 0:cw], ctiles[pr][:, :, coff:coff + cw], AF.Tanh)

            def h_out(q):
                # h = o * tanh(c); then refresh htT for the next wave
                pr, qh = q // 2, q % 2
                hsl = slice(qh * nchq, (qh + 1) * nchq)
                nc.vector.tensor_mul(
                    htmps[pr][:, hsl, coff:coff + cw], sig_l[q][:, :, 2 * gw:3 * gw],
                    tct_l[pr][:, hsl, 0:cw],
                )
                if s < seq:
                    nc.vector.transpose(
                        htTs[pr][:, hsl, :].rearrange("p c k -> p (c k)"),
                        htmps[pr][:, hsl, :].rearrange("p c k -> p (c k)"),
                    )

            def z_out(pr):
                # ship raw h1 (bf16); the MLP head applies tanh on load
                nc.vector.tensor_copy(
                    zaccs[pr][:, (s - 1) % ZB, :, :], htmps[pr][:, :, 10:20]
                )

            sigma(0)
            pool_chain(0)
            sigma(1)
            pool_chain(1)
            sigma(2)
            pool_chain(2)
            tanh_c(0)
            h_out(0)
            h_out(1)
            sigma(3)
            pool_chain(3)
            if L1:
                z_out(0)
            tanh_c(1)
            h_out(2)
            h_out(3)
            if L1:
                z_out(1)
            if L1:
                t = s - 1
                if t % ZB == ZB - 1 or s == seq:
                    nzb = t % ZB + 1
                    t0 = t - nzb + 1
                    for pr in range(npair):
                        dst = z_d[t0:t0 + nzb].rearrange(
                            "t p (c h) -> p t c h", h=H
                        )[:, :, pr * nchp:(pr + 1) * nchp, :]
                        nc.sync.dma_start(dst, zaccs[pr][:, 0:nzb, :, :])
    return nc


def build_mlp():
    """out2 = sigmoid(Z2 @ W1.T + b1) @ W2.T + b2 for one row-shard.

    z2t carries raw bf16 h1 values; tanh is applied on load into an f32r
    tile so the matmul products keep full precision.  The b1 bias enters
    through an 8th contraction chunk whose rhs is a constant ones row.
    """
    nc = bass.Bass("TRN2")
    z2t_d = nc.declare_dram_parameter("z2t", [K2, 128, BC], BF16, isOutput=False)
    w1b_d = nc.declare_dram_parameter("w1b", [K2 + 1, 128, H], F32R, isOutput=False)
    w2b_d = nc.declare_dram_parameter("w2b", [H + 1, 40], F32R, isOutput=False)
    ones_d = nc.declare_dram_parameter("ones", [1, BC], F32R, isOutput=False)
    onesc_d = nc.declare_dram_parameter("onesc", [128, 512], F32R, isOutput=False)
    out_d = nc.declare_dram_parameter("out2", [40, BC], F32, isOutput=True)

    with PatchedTileContext(nc) as tc, ExitStack() as ctx:
        const = ctx.enter_context(tc.tile_pool(name="const", bufs=1))
        pool = ctx.enter_context(tc.tile_pool(name="pool", bufs=3))
        ps = ctx.enter_context(tc.tile_pool(name="ps", bufs=2, space="PSUM"))

        w1 = const.tile([128, K2 + 1, H], F32R, name="w1_t")
        nc.sync.dma_start(w1[:], w1b_d[:].rearrange("k p h -> p k h"))
        w2 = const.tile([H + 1, 40], F32R, name="w2_t")
        nc.sync.dma_start(w2[:], w2b_d[:])
        sstack = const.tile([H + 1, BC], F32R, name="sstack")
        nc.sync.dma_start(sstack[H:H + 1, :], ones_d[:])
        onest = const.tile([128, 512], F32R, name="onest")
        nc.sync.dma_start(onest[:], onesc_d[:])

        NCOL = BC // 512
        for col in range(NCOL):
            zz = pool.tile([128, K2, 512], BF16, name="zz", tag="zz")
            nc.sync.dma_start(
                zz[:], z2t_d[:, :, col * 512:(col + 1) * 512].rearrange("k p n -> p k n")
            )
            zz2 = pool.tile([128, K2, 512], F32R, name="zz2", tag="zz2")
            nc.scalar.activation(zz2[:], zz[:], AF.Tanh)
            a1 = ps.tile([H, 512], F32, name="a1", tag="a1")
            for k in range(K2):
                nc.tensor.matmul(
                    a1[:], w1[:, k, :], zz2[:, k, :], start=(k == 0), stop=False
                )
            nc.tensor.matmul(a1[:], w1[:, K2, :], onest[:], start=False, stop=True)
            nc.scalar.activation(sstack[0:H, col * 512:(col + 1) * 512], a1[:], AF.Sigmoid)
        for col in range(NCOL):
            a2 = ps.tile([40, 512], F32, name="a2", tag="a2")
            nc.tensor.matmul(a2[:], w2[:], sstack[:, col * 512:(col + 1) * 512], start=True, stop=True)
            ot = pool.tile([40, 512], F32, name="ot", tag="ot")
            nc.vector.tensor_copy(ot[:], a2[:])
            nc.sync.dma_start(out_d[:, col * 512:(col + 1) * 512], ot[:])
    return nc


def _build_weight_mats(Wih0, Whh0, bih0, bhh0, Wih1, Whh1, bih1, bhh1):
    """[23, ncols] combined weight blocks, replicated at partitions 0/32/64/96."""
    b0 = (bih0 + bhh0).astype(np.float32)
    b1 = (bih1 + bhh1).astype(np.float32)
    rows = {"i": slice(0, 10), "f": slice(10, 20), "g": slice(20, 30), "o": slice(30, 40)}
    order = ["i", "f", "o", "g"]
    wmid = np.zeros((KR, 80), np.float32)
    wfirst = np.zeros((KR, 40), np.float32)
    wlast = np.zeros((KR, 40), np.float32)
    for bi, gtp in enumerate(order):
        gr = rows[gtp]
        c0 = slice(bi * 20, bi * 20 + 10)
        c1 = slice(bi * 20 + 10, bi * 20 + 20)
        wmid[0:10, c0] = Whh0[gr, :].T
        wmid[20:22, c0] = Wih0[gr, :].T
        wmid[22, c0] = b0[gr]
        wmid[0:10, c1] = Wih1[gr, :].T
        wmid[10:20, c1] = Whh1[gr, :].T
        wmid[22, c1] = b1[gr]
        cs = slice(bi * 10, bi * 10 + 10)
        wfirst[0:10, cs] = Whh0[gr, :].T
        wfirst[20:22, cs] = Wih0[gr, :].T
        wfirst[22, cs] = b0[gr]
        wlast[0:10, cs] = Wih1[gr, :].T
        wlast[10:20, cs] = Whh1[gr, :].T
        wlast[22, cs] = b1[gr]

    # pre-double the g-columns: sigmoid(2*g) = (tanh(g)+1)/2
    wmid[:, 60:80] *= 2.0
    wfirst[:, 30:40] *= 2.0
    wlast[:, 30:40] *= 2.0

    def rep4(w):
        out = np.zeros((128, w.shape[1]), np.float32)
        for i in range(4):
            out[32 * i:32 * i + KR, :] = w
        return out

    return rep4(wfirst), rep4(wmid), rep4(wlast)


_CACHE = {}


def _get_lstm():
    if "lstm" not in _CACHE:
        _CACHE["lstm"] = build_lstm()
    return _CACHE["lstm"]


def _get_mlp():
    if "mlp" not in _CACHE:
        _CACHE["mlp"] = build_mlp()
    return _CACHE["mlp"]


def _batch_layout(v2):
    """[BC, CW] -> [128, BC//128, CW] with b = 128*c + p."""
    return np.ascontiguousarray(v2.reshape(BC // 128, 128, CW).transpose(1, 0, 2))


def kernel(x, h0, c0, Wih0, Whh0, bih0, bhh0, Wih1, Whh1, bih1, bhh1, W1, b1, W2, b2):
    x = np.asarray(x, np.float32)
    h0 = np.asarray(h0, np.float32)
    c0 = np.asarray(c0, np.float32)
    wfirst, wmid, wlast = _build_weight_mats(
        np.asarray(Wih0, np.float32), np.asarray(Whh0, np.float32),
        np.asarray(bih0, np.float32), np.asarray(bhh0, np.float32),
        np.asarray(Wih1, np.float32), np.asarray(Whh1, np.float32),
        np.asarray(bih1, np.float32), np.asarray(bhh1, np.float32),
    )
    wfirst, wmid, wlast = (w.astype(NPBF) for w in (wfirst, wmid, wlast))
    core_ids = list(range(NCORES))

    in_maps = []
    for j in core_ids:
        bsl = slice(j * BC, (j + 1) * BC)
        xT = np.ascontiguousarray(
            x[:, bsl, :].reshape(SEQ, BC // 128, 128, IN).transpose(0, 2, 1, 3)
        ).astype(NPBF)
        h0p = _batch_layout(np.concatenate([h0[0, bsl, :], h0[1, bsl, :]], axis=1)).astype(NPBF)
        c0p = _batch_layout(np.concatenate([c0[0, bsl, :], c0[1, bsl, :]], axis=1))
        in_maps.append({
            "xT": xT, "h0p": h0p, "c0p": c0p,
            "wfirst": wfirst, "wmid": wmid, "wlast": wlast,
        })

    res1 = run_bass_kernel_spmd(_get_lstm(), in_maps, core_ids).results

    # z dram layout per core: [t, p, c*H + h] with local batch b = 128*c + p
    # (carries raw bf16 h1; the MLP kernel applies tanh on load)
    z_cores = []
    for j in core_ids:
        zj = res1[j]["z"].reshape(SEQ, 128, BC // 128, H).transpose(0, 2, 1, 3)
        z_cores.append(zj.reshape(SEQ, BC, H))
    z_global = np.concatenate(z_cores, axis=1)          # [T, B, H] bf16
    Z2 = np.ascontiguousarray(z_global).reshape(B, SEQ * H)

    w1b = np.zeros(((K2 + 1) * 128, H), np.float32)
    w1b[0:SEQ * H, :] = np.asarray(W1, np.float32).T
    w1b[K2 * 128, :] = np.asarray(b1, np.float32)
    w1b = w1b.reshape(K2 + 1, 128, H)
    w2b = np.zeros((H + 1, 40), np.float32)
    w2b[0:H, :] = np.asarray(W2, np.float32).T
    w2b[H, :] = np.asarray(b2, np.float32)

    onesc = np.zeros((128, 512), np.float32)
    onesc[0, :] = 1.0
    in_maps2 = []
    for j in core_ids:
        rows = slice(j * BC, (j + 1) * BC)
        z2t = np.zeros((K2 * 128, BC), NPBF)
        z2t[0:SEQ * H, :] = Z2[rows, :].T
        in_maps2.append({
            "z2t": np.ascontiguousarray(z2t.reshape(K2, 128, BC)),
            "w1b": w1b, "w2b": w2b,
            "ones": np.ones((1, BC), np.float32),
            "onesc": onesc,
        })

    res2 = run_bass_kernel_spmd(_get_mlp(), in_maps2, core_ids).results
    out2 = np.concatenate([res2[j]["out2"] for j in core_ids], axis=1)  # [40, B]
    out = np.ascontiguousarray(out2.T).reshape(OUT_LEN, B, OUT_SIZE)
    return out
